# revision 3
# baseline (speedup 1.0000x reference)
"""DotGAT (2-layer dot-product graph attention) on 8 Trainium2 NeuronCores.

V2 design:
- per-(core,half) degree-sorted padded-CSR edge phases (as baseline)
- scatter-add of per-batch segment results into node-id-ordered hn tables
  (merges become plain loads; no pos-gathers, no cross-half add)
- DVE 4D-broadcast attention-weight multiply (no gpsimd exr materialize)
- hybrid fold+reduce aggregation over k
- W2 projection fused into merge1 via PE transpose + matmul (no proj2 phase)
- bf16 [NV_PAD,64] AllGather + pad-expand into gatherable 256B-row tables
- batched f1c writes, bf16 hT loads
"""

import sys

sys.path.insert(0, "/opt/trn_rl_repo")

from contextlib import ExitStack

import numpy as np
import ml_dtypes

import concourse.bass as bass
import concourse.bacc as bacc
import concourse.mybir as mybir
from concourse.tile import TileContext

bf = ml_dtypes.bfloat16


N = 50000
E = 1600000
NCORES = 8
NPC = N // NCORES  # 6250 nodes per core
CHUNK = 25000
SENT = CHUNK  # sentinel row in f1c chunk tables
NT = 49  # tiles of 128 virtual nodes per (core, half)
NV_PAD = NT * 128  # 6272
DUMP = 6256  # dump row (node-id space) for padded scatter rows
SENTL = 6260  # zero row (node-id space) for L2 gathers
HALF2 = NV_PAD * 4  # 25088 rows per half in the f2 tables

dt = mybir.dt
F32, BF16, I16 = dt.float32, dt.bfloat16, dt.int16
AX = mybir.AxisListType
OP = mybir.AluOpType
AF = mybir.ActivationFunctionType

HEADS, HID, D1, D2 = 8, 16, 128, 64
SC1, SC2 = float(HID**-0.5), float(D2**-0.5)
BATCH_CAP = 64  # max slots (b*K) per fs gather batch
MERGE_B = 7  # merge tiles per iteration (49 = 7*7)


def wrap16(idx):
    """int array [S] -> int16 [128, S//16] wrapped+replicated gather layout."""
    S = len(idx)
    assert S % 16 == 0
    w = np.asarray(idx, np.int32).reshape(S // 16, 16).T.astype(np.int16)
    return np.tile(w, (8, 1))


def prep(src, dst):
    """Returns (sched, [per-core data dicts])."""
    src = np.asarray(src, np.int64)
    dst = np.asarray(dst, np.int64)
    core_of = dst // NPC
    half_of = (src >= CHUNK).astype(np.int64)

    order = np.lexsort((dst, half_of, core_of))
    s_src = src[order]
    s_dst = dst[order]
    s_core = core_of[order]
    s_half = half_of[order]

    percore = []
    info = {}
    for c in range(NCORES):
        for h in range(2):
            m = (s_core == c) & (s_half == h)
            esrc = s_src[m]
            edst = s_dst[m] - c * NPC
            deg = np.bincount(edst, minlength=NPC)
            vnodes = np.nonzero(deg > 0)[0]
            nv = len(vnodes)
            assert nv <= NV_PAD, (c, h, nv)
            vorder = vnodes[np.argsort(-deg[vnodes], kind="stable")]
            vdeg = deg[vorder]
            ks = np.zeros(NT, np.int64)
            for t in range(NT):
                seg = vdeg[t * 128 : (t + 1) * 128]
                ks[t] = seg[0] if len(seg) else 1
            ks = np.maximum(ks, 1)
            info[(c, h)] = dict(
                esrc=esrc, edst=edst, deg=deg, vorder=vorder, vdeg=vdeg, ks=ks
            )

    # shared schedule: per half, per tile, max K over cores; batch consecutive
    # tiles into equal-K groups with b*K <= BATCH_CAP and b <= 8.
    sched = {}
    for h in range(2):
        ks_raw = np.max([info[(c, h)]["ks"] for c in range(NCORES)], axis=0)
        batches = []  # (t0, b, K, slot_off)
        t = 0
        off = 0
        ks_pad = np.zeros(NT, np.int64)
        while t < NT:
            K = int(ks_raw[t])
            assert K <= BATCH_CAP, f"tile degree {K} exceeds BATCH_CAP"
            b = 1
            while (
                t + b < NT
                and b < 4
                and (b + 1) * K <= BATCH_CAP
                and ks_raw[t + b] > K - 3
            ):
                b += 1
            ks_pad[t : t + b] = K
            batches.append((t, b, K, off))
            off += 128 * K * b
            t += b
        sched[h] = {"ks": ks_pad, "batches": batches}

    for c in range(NCORES):
        data = {}
        for h in range(2):
            d = info[(c, h)]
            ks = sched[h]["ks"]
            vorder, vdeg = d["vorder"], d["vdeg"]
            starts = np.zeros(NPC + 1, np.int64)
            np.cumsum(np.bincount(d["edst"], minlength=NPC), out=starts[1:])
            src_slots = []
            npad = np.zeros((128, NT), np.float32)
            fd0 = np.full((NT, 128), SENT, np.int64)
            fd1 = np.full((NT, 128), SENT, np.int64)
            fdl = np.full((NT, 128), SENTL, np.int64)  # L2 fd (local rows)
            sct = np.full((NT, 128), DUMP, np.int64)  # scatter rows (node ids)
            for t in range(NT):
                K = ks[t]
                tile_nodes = vorder[t * 128 : (t + 1) * 128]
                tn = len(tile_nodes)
                slot = np.full((128, K), -1, np.int64)
                for p in range(tn):
                    n = tile_nodes[p]
                    dg = vdeg[t * 128 + p]
                    e0 = starts[n]
                    slot[p, :dg] = d["esrc"][e0 : e0 + dg]
                    npad[p, t] = K - dg
                npad[tn:, t] = K
                gnodes = tile_nodes + c * NPC
                in0 = gnodes < CHUNK
                fd0[t, :tn] = np.where(in0[:tn], gnodes[:tn], SENT)
                fd1[t, :tn] = np.where(~in0[:tn], gnodes[:tn] - CHUNK, SENT)
                fdl[t, :tn] = tile_nodes[:tn]
                sct[t, :tn] = tile_nodes[:tn]
                src_slots.append(slot.T.reshape(-1))  # [K*128] k-major
            allslots = np.concatenate(src_slots)
            pad = allslots < 0
            l1 = np.where(pad, SENT, allslots - CHUNK * h)
            rem = np.where(pad, 0, allslots)
            l2 = NV_PAD * (rem // NPC) + rem % NPC - HALF2 * h
            l2 = np.where(pad, SENTL, l2)
            data[f"h{h}"] = dict(
                src1=l1,
                src2=l2,
                npad=npad,
                fd0=fd0.reshape(-1),
                fd1=fd1.reshape(-1),
                fdl=fdl.reshape(-1),
                sct=sct.reshape(-1),
            )
        percore.append(data)
    return sched, percore


def build(sched):
    nc = bacc.Bacc("TRN2", target_bir_lowering=False, debug=False, num_devices=8)

    S = {h: int(128 * np.sum(sched[h]["ks"])) for h in (0, 1)}

    # ---------------- I/O ----------------
    hTb = nc.declare_dram_parameter("hTb", [128, N], BF16, isOutput=False)
    W1 = nc.declare_dram_parameter("W1", [128, D1], BF16, isOutput=False)
    W2b = nc.declare_dram_parameter("W2b", [128, D2], BF16, isOutput=False)
    ident = nc.declare_dram_parameter("ident", [128, 128], BF16, isOutput=False)
    ins = {}
    for h in (0, 1):
        ins[f"src1_{h}"] = nc.declare_dram_parameter(
            f"src1_{h}", [128, S[h] // 16], I16, isOutput=False
        )
        ins[f"src2_{h}"] = nc.declare_dram_parameter(
            f"src2_{h}", [128, S[h] // 16], I16, isOutput=False
        )
        for nm in ("fd0", "fd1", "fdl", "sct"):
            ins[f"{nm}_{h}"] = nc.declare_dram_parameter(
                f"{nm}_{h}", [128, NT * 8], I16, isOutput=False
            )
        ins[f"npad_{h}"] = nc.declare_dram_parameter(
            f"npad_{h}", [128, NT], F32, isOutput=False
        )
    out = nc.declare_dram_parameter("out", [NV_PAD, D2], F32, isOutput=True)

    with ExitStack() as ctx:
        tc = ctx.enter_context(TileContext(nc))
        dram = ctx.enter_context(tc.tile_pool(name="dram", bufs=1, space="DRAM"))
        f1c = [dram.tile([CHUNK + 1, D1], BF16, tag=f"f1c{i}", name=f"f1c{i}") for i in range(2)]
        hn1 = dram.tile([NV_PAD, 192], F32, name="hn1")
        hn2 = dram.tile([NV_PAD, 128], F32, name="hn2")
        f2my = dram.tile([NV_PAD, 128], BF16, name="f2my")
        f2cmp = dram.tile([NV_PAD, D2], BF16, name="f2cmp")
        f2gath = dram.tile([NV_PAD * NCORES, D2], BF16, name="f2gath")
        f2tab = [
            dram.tile([HALF2, 128], BF16, name=f"f2tab{i}") for i in range(2)
        ]

        consts = ctx.enter_context(tc.tile_pool(name="consts", bufs=1))
        w1t = consts.tile([128, D1], BF16)
        nc.sync.dma_start(out=w1t[:, :], in_=W1[:, :])
        w2t = consts.tile([128, D2], BF16)
        nc.sync.dma_start(out=w2t[:, :], in_=W2b[:, :])
        idt = consts.tile([128, 128], BF16)
        nc.sync.dma_start(out=idt[:, :], in_=ident[:, :])
        zrow_bf = consts.tile([128, 128], BF16)
        nc.gpsimd.memset(zrow_bf[:, :], 0.0)
        ztile = consts.tile([128, 8, 192], F32)
        nc.gpsimd.memset(ztile[:, :, :], 0.0)

        # sentinel zero rows for f1c tables
        nc.sync.dma_start(out=f1c[0][CHUNK : CHUNK + 1, :], in_=zrow_bf[:1, :D1])
        nc.sync.dma_start(out=f1c[1][CHUNK : CHUNK + 1, :], in_=zrow_bf[:1, :D1])

        def zero_table(tab, w):
            for t in range(0, NT, 8):
                b = min(8, NT - t)
                nc.sync.dma_start(
                    out=tab[:, :w].rearrange("(t p) w -> p t w", p=128)[
                        :, t : t + b, :
                    ],
                    in_=ztile[:, :b, :w],
                )

        # ------------- Phase 0: feat1 = h @ W1 -> bf16 chunk tables ----------
        def phase0_block(t, p0, p0ps):
            n0 = t * 1024
            bn = min(1024, N - n0)
            lt = p0.tile([128, 1024], BF16, tag="lhst", name="lhst")
            nc.gpsimd.dma_start(out=lt[:, :bn], in_=hTb[:, n0 : n0 + bn])
            ps = p0ps.tile([128, 1024], F32, name="ps0")
            nsub = (bn + 127) // 128
            for s in range(nsub):
                sn = min(128, bn - 128 * s)
                nc.tensor.matmul(
                    ps[:sn, 128 * s : 128 * s + 128],
                    lt[:, 128 * s : 128 * s + sn],
                    w1t[:, :],
                    start=True,
                    stop=True,
                )
            ob = p0.tile([128, 1024], BF16, tag="f1out", name="f1out")
            nc.scalar.activation(ob[:, : 128 * nsub], ps[:, : 128 * nsub], AF.Copy)
            if bn % 128 == 0 and (n0 + bn <= CHUNK or n0 >= CHUNK):
                ci = 0 if n0 < CHUNK else 1
                r0 = n0 - CHUNK * ci
                nc.sync.dma_start(
                    out=f1c[ci][r0 : r0 + bn, :].rearrange(
                        "(s p) c -> p s c", p=128
                    ),
                    in_=ob[:, :bn].rearrange("p (s c) -> p s c", c=128),
                )
            else:
                # straddling block: per-128-row writes with partition splits
                for s in range(nsub):
                    sn = min(128, bn - 128 * s)
                    r0 = n0 + 128 * s
                    obs = ob[:sn, 128 * s : 128 * s + 128]
                    if r0 + sn <= CHUNK:
                        nc.sync.dma_start(out=f1c[0][r0 : r0 + sn, :], in_=obs)
                    elif r0 >= CHUNK:
                        nc.sync.dma_start(
                            out=f1c[1][r0 - CHUNK : r0 - CHUNK + sn, :], in_=obs
                        )
                    else:
                        a = CHUNK - r0
                        nc.sync.dma_start(
                            out=f1c[0][r0 : r0 + a, :],
                            in_=ob[:a, 128 * s : 128 * s + 128],
                        )
                        nc.sync.dma_start(
                            out=f1c[1][0 : sn - a, :],
                            in_=ob[a:sn, 128 * s : 128 * s + 128],
                        )

        # ---------------- edge phase (shared for both layers) ----------------
        def edge_prep(layer, hh, ep):
            # FD tile per half (shared across layers)
            fdt = ep.tile([128, NT, 128], BF16, tag=f"fdt{hh}", name=f"fdt{hh}")
            if layer == 1:
                # fge/fdi shared across halves: freed after the quick add
                fge = ep.tile([128, NT, 128], BF16, tag="fdg", name="fdg")
                fdi = [
                    ep.tile([128, NT * 8], I16, tag=f"fdi_{i}", name=f"fdi_{i}")
                    for i in range(2)
                ]
                for i, fn in enumerate(("fd0", "fd1")):
                    nc.sync.dma_start(out=fdi[i][:, :], in_=ins[f"{fn}_{hh}"][:, :])
                nc.gpsimd.dma_gather(
                    out_ap=fdt[:, :, :],
                    in_ap=f1c[0][:, :],
                    idxs_ap=fdi[0][:, :],
                    num_idxs=NT * 128,
                    num_idxs_reg=NT * 128,
                    elem_size=128,
                    single_packet=False,
                )
                nc.gpsimd.dma_gather(
                    out_ap=fge[:, :, :],
                    in_ap=f1c[1][:, :],
                    idxs_ap=fdi[1][:, :],
                    num_idxs=NT * 128,
                    num_idxs_reg=NT * 128,
                    elem_size=128,
                    single_packet=False,
                )
                nc.vector.tensor_tensor(
                    out=fdt[:, :, :], in0=fdt[:, :, :], in1=fge[:, :, :], op=OP.add
                )
            else:
                fdi = ep.tile([128, NT * 8], I16, tag="fdi_0", name="fdi_0")
                nc.sync.dma_start(out=fdi[:, :], in_=ins[f"fdl_{hh}"][:, :])
                nc.gpsimd.dma_gather(
                    out_ap=fdt[:, :, :],
                    in_ap=f2my[:, :],
                    idxs_ap=fdi[:, :],
                    num_idxs=NT * 128,
                    num_idxs_reg=NT * 128,
                    elem_size=128,
                    single_packet=False,
                )
            npt = ep.tile([128, NT], F32, tag=f"npad{hh}", name=f"npad{hh}")
            nc.sync.dma_start(out=npt[:, :], in_=ins[f"npad_{hh}"][:, :])
            sci = ep.tile([128, NT * 8], I16, tag=f"sct{hh}", name=f"sct{hh}")
            nc.sync.dma_start(out=sci[:, :], in_=ins[f"sct_{hh}"][:, :])
            return fdt, npt, sci

        def edge_batches(layer, hh, state, eps, epf, epi, aggps):
            fdt, npt, sci = state
            D = D1 if layer == 1 else D2
            H = HEADS if layer == 1 else 1
            C = D // H
            scale = SC1 if layer == 1 else SC2
            srcname = f"src{layer}_{hh}"
            hntab = hn1 if layer == 1 else hn2
            stg_w = 192 if layer == 1 else 128
            den0 = D  # denominator column offset within stg
            batches = sched[hh]["batches"]

            ftab = f1c[hh][:, :] if layer == 1 else f2tab[hh][:, :]

            # software-pipelined batch loop (3 skewed stages):
            #   stage0(i): idx load + fs gather          [SP/Pool/DMA]
            #   stage1(i): prod, fold, exp               [DVE, Act]
            #   stage2(i): den, wa, agg, scatter         [DVE, Pool]
            # Issue order per iteration: s0(i+1), s1(i), s2(i-1) so the DVE
            # never head-of-line blocks on a gather or on the Act exp.
            live = {}

            def s0(i):
                t0, b, K, off = batches[i]
                bK = b * K
                it = epi.tile([128, 8 * BATCH_CAP], I16, tag="srcidx")
                nc.sync.dma_start(
                    out=it[:, : 8 * bK],
                    in_=ins[srcname][:, off // 16 : off // 16 + 8 * bK],
                )
                fsg = epf.tile([128, BATCH_CAP, 128], BF16, tag="fsg")
                nc.gpsimd.dma_gather(
                    out_ap=fsg[:, :bK, :],
                    in_ap=ftab,
                    idxs_ap=it[:, : 8 * bK],
                    num_idxs=128 * bK,
                    num_idxs_reg=128 * bK,
                    elem_size=128,
                    single_packet=False,
                )
                live[i] = {"fsg": fsg}

            def s1(i):
                t0, b, K, off = batches[i]
                bK = b * K
                fsg = live[i]["fsg"]
                prod = eps.tile([128, BATCH_CAP, D1], BF16, tag="pw")
                for j in range(b):
                    nc.vector.tensor_tensor(
                        out=prod[:, j * K : (j + 1) * K, :D],
                        in0=fsg[:, j * K : (j + 1) * K, :D],
                        in1=fdt[:, t0 + j, :D].unsqueeze(1).broadcast_to([128, K, D]),
                        op=OP.mult,
                    )
                cur = D
                while cur > H:
                    half = cur // 2
                    nc.vector.tensor_tensor(
                        out=prod[:, :bK, :half],
                        in0=prod[:, :bK, :half],
                        in1=prod[:, :bK, half:cur],
                        op=OP.add,
                    )
                    cur = half
                ex = eps.tile([128, BATCH_CAP * HEADS], BF16, tag="ex")
                exv = ex[:, : bK * H].rearrange("p (k h) -> p k h", k=bK)
                nc.scalar.activation(exv, prod[:, :bK, :H], AF.Exp, scale=scale)
                live[i]["ex"] = ex

            def s2(i):
                t0, b, K, off = batches[i]
                bK = b * K
                fsg = live[i]["fsg"]
                ex = live[i]["ex"]
                stg = eps.tile([128, 8 * 192], F32, tag="stg")
                stg4 = stg[:, : b * stg_w].rearrange("p (b w) -> p b w", b=b)
                if i < 2:
                    # zero the pad columns once per rotating buffer; later
                    # batches never write them so they stay zero
                    nc.gpsimd.memset(
                        stg[:, :].rearrange("p (b w) -> p b w", w=stg_w)[
                            :, :, den0 + H :
                        ],
                        0.0,
                    )
                nc.vector.tensor_reduce(
                    out=stg4[:, :, den0 : den0 + H],
                    in_=ex[:, : bK * H].rearrange("p (b k h) -> p b h k", b=b, k=K),
                    axis=AX.X,
                    op=OP.add,
                )
                nc.vector.tensor_tensor(
                    out=stg4[:, :, den0 : den0 + H],
                    in0=stg4[:, :, den0 : den0 + H],
                    in1=npt[:, t0 : t0 + b].unsqueeze(2).broadcast_to([128, b, H]),
                    op=OP.subtract,
                )
                # weighted features in-place: fsg *= ex (broadcast over c).
                # Split ~1/4 of the slots onto the Pool engine to offload DVE.
                if H > 1:
                    kp = (bK // 4) & ~1
                    if kp >= 8:
                        nc.gpsimd.tensor_tensor(
                            out=fsg[:, :kp, :D].rearrange(
                                "p k (c h) -> p k c h", c=C
                            ),
                            in0=fsg[:, :kp, :D].rearrange(
                                "p k (c h) -> p k c h", c=C
                            ),
                            in1=ex[:, : kp * H]
                            .rearrange("p (k h) -> p k h", k=kp)
                            .unsqueeze(2)
                            .broadcast_to([128, kp, C, H]),
                            op=OP.mult,
                        )
                    else:
                        kp = 0
                    nc.vector.tensor_tensor(
                        out=fsg[:, kp:bK, :D].rearrange(
                            "p k (c h) -> p k c h", c=C
                        ),
                        in0=fsg[:, kp:bK, :D].rearrange(
                            "p k (c h) -> p k c h", c=C
                        ),
                        in1=ex[:, kp * H : bK * H]
                        .rearrange("p (k h) -> p k h", k=bK - kp)
                        .unsqueeze(2)
                        .broadcast_to([128, bK - kp, C, H]),
                        op=OP.mult,
                    )
                else:
                    nc.vector.tensor_tensor(
                        out=fsg[:, :bK, :D],
                        in0=fsg[:, :bK, :D],
                        in1=ex[:, :bK].unsqueeze(2).broadcast_to([128, bK, D]),
                        op=OP.mult,
                    )
                if layer == 1:
                    # aggregation over k on the PE: transpose-accumulate each
                    # weighted slot into PSUM (f32), then transpose back.
                    aggT = aggps.tile([128, 512], F32, name="aggT")
                    for j in range(b):
                        for k in range(K):
                            nc.tensor.matmul(
                                aggT[:, j * 128 : (j + 1) * 128],
                                fsg[:, j * K + k, :],
                                idt[:, :],
                                start=(k == 0),
                                stop=(k == K - 1),
                            )
                    aggTs = eps.tile([128, 512], BF16, tag="aggTs")
                    nc.scalar.activation(
                        aggTs[:, : b * 128], aggT[:, : b * 128], AF.Copy
                    )
                    nodeT = aggps.tile([128, 512], F32, name="nodeT")
                    for j in range(b):
                        nc.tensor.matmul(
                            nodeT[:, j * 128 : (j + 1) * 128],
                            aggTs[:, j * 128 : (j + 1) * 128],
                            idt[:, :],
                            start=True,
                            stop=True,
                        )
                    nc.scalar.activation(
                        stg4[:, :, :D],
                        nodeT[:, : b * 128].rearrange("p (b d) -> p b d", b=b),
                        AF.Copy,
                    )
                else:
                    # aggregation over k: fold (bf16) then reduce (f32 accum)
                    v = fsg[:, :bK, :D].rearrange("p (b k) d -> p b k d", b=b)
                    cur = K
                    folds = 0
                    while folds < 2 and cur > 4:
                        if cur % 2:
                            nc.vector.tensor_tensor(
                                out=v[:, :, 0:1, :D],
                                in0=v[:, :, 0:1, :D],
                                in1=v[:, :, cur - 1 : cur, :D],
                                op=OP.add,
                            )
                            cur -= 1
                        h2 = cur // 2
                        nc.vector.tensor_tensor(
                            out=v[:, :, :h2, :D],
                            in0=v[:, :, :h2, :D],
                            in1=v[:, :, h2:cur, :D],
                            op=OP.add,
                        )
                        cur = h2
                        folds += 1
                    nc.vector.tensor_reduce(
                        out=stg4[:, :, :D],
                        in_=v[:, :, :cur, :D].rearrange("p b k d -> p b d k"),
                        axis=AX.X,
                        op=OP.add,
                    )
                nc.gpsimd.dma_scatter_add(
                    out_ap=hntab[:, :stg_w],
                    in_ap=stg4[:, :, :],
                    idxs_ap=sci[:, 8 * t0 : 8 * (t0 + b)],
                    num_idxs=128 * b,
                    num_idxs_reg=128 * b,
                    elem_size=stg_w,
                    single_packet=False,
                )
                del live[i]

            nbat = len(batches)
            s0(0)
            if nbat > 1:
                s0(1)
            for i in range(nbat):
                if i + 2 < nbat:
                    s0(i + 2)
                s1(i)
                if i >= 1:
                    s2(i - 1)
            s2(nbat - 1)

        # ---------------- merge1 + local W2 projection ----------------
        def merge1(mp, mps, mb_lo, mb_hi):
            for mb in range(mb_lo, mb_hi, MERGE_B):
                b = min(MERGE_B, NT - mb)
                g = mp.tile([128, MERGE_B, 136], F32, tag="g1", name="g1")
                nc.sync.dma_start(
                    out=g[:, :b, :],
                    in_=hn1[:, :136].rearrange("(t p) w -> p t w", p=128)[
                        :, mb : mb + b, :
                    ],
                )
                den = mp.tile([128, MERGE_B * 8], F32, tag="den")
                nc.vector.tensor_scalar_max(den[:, : b * 8], g[:, :b, 128:136], 1e-9)
                rcp = mp.tile([128, MERGE_B * 8], F32, tag="rcp")
                nc.vector.reciprocal(rcp[:, : b * 8], den[:, : b * 8])
                o1 = mp.tile([128, MERGE_B, 128], F32, tag="o1")
                nc.vector.tensor_tensor(
                    out=o1[:, :b, :].rearrange("p b (c h) -> p b c h", c=HID),
                    in0=g[:, :b, :128].rearrange("p b (c h) -> p b c h", c=HID),
                    in1=rcp[:, : b * 8]
                    .rearrange("p (b h) -> p b h", b=b)
                    .unsqueeze(2)
                    .broadcast_to([128, b, HID, HEADS]),
                    op=OP.mult,
                )
                # ELU -> bf16 (2-byte ops get 2x DVE throughput)
                mx = mp.tile([128, MERGE_B, 128], BF16, tag="mx")
                nc.vector.tensor_scalar_max(mx[:, :b, :], o1[:, :b, :], 0.0)
                mn = mp.tile([128, MERGE_B, 128], BF16, tag="mn")
                nc.vector.tensor_scalar_min(mn[:, :b, :], o1[:, :b, :], 0.0)
                e = mp.tile([128, MERGE_B, 128], BF16, tag="e")
                nc.scalar.activation(e[:, :b, :], mn[:, :b, :], AF.Exp)
                nc.vector.tensor_scalar_add(e[:, :b, :], e[:, :b, :], -1.0)
                h1t = mp.tile([128, MERGE_B, 128], BF16, tag="h1t")
                nc.vector.tensor_tensor(
                    out=h1t[:, :b, :], in0=mx[:, :b, :], in1=e[:, :b, :], op=OP.add
                )
                # fused projection: f2 = h1 @ W2 per 128-node tile
                f2p = mp.tile([128, MERGE_B, 128], BF16, tag="f2p")
                nc.gpsimd.memset(f2p[:, :b, D2:], 0.0)
                for j in range(b):
                    psT = mps.tile([128, 128], BF16, name="psT")
                    nc.tensor.matmul(
                        psT[:, :], h1t[:, j, :], idt[:, :],
                        start=True, stop=True, is_transpose=True,
                    )
                    h1T = mp.tile([128, 128], BF16, tag="h1T")
                    nc.scalar.activation(h1T[:, :], psT[:, :], AF.Copy)
                    ps2 = mps.tile([128, D2], F32, name="ps2")
                    nc.tensor.matmul(
                        ps2[:, :], h1T[:, :], w2t[:, :], start=True, stop=True
                    )
                    nc.scalar.activation(f2p[:, j, :D2], ps2[:, :], AF.Copy)
                nc.sync.dma_start(
                    out=f2my[:, :].rearrange("(t p) c -> p t c", p=128)[
                        :, mb : mb + b, :
                    ],
                    in_=f2p[:, :b, :],
                )
                nc.sync.dma_start(
                    out=f2cmp[:, :].rearrange("(t p) c -> p t c", p=128)[
                        :, mb : mb + b, :
                    ],
                    in_=f2p[:, :b, :D2],
                )

        # ---------------- merge2 -> output ----------------
        def merge2(mp):
            for mb in range(0, NT, MERGE_B):
                b = min(MERGE_B, NT - mb)
                g = mp.tile([128, MERGE_B, 65], F32, tag="g2", name="g2")
                nc.sync.dma_start(
                    out=g[:, :b, :],
                    in_=hn2[:, :65].rearrange("(t p) w -> p t w", p=128)[
                        :, mb : mb + b, :
                    ],
                )
                den = mp.tile([128, MERGE_B], F32, tag="den2")
                nc.vector.tensor_scalar_max(den[:, :b], g[:, :b, 64:65], 1e-9)
                rcp = mp.tile([128, MERGE_B], F32, tag="rcp2")
                nc.vector.reciprocal(rcp[:, :b], den[:, :b])
                o2 = mp.tile([128, MERGE_B, D2], F32, tag="o2")
                nc.vector.tensor_tensor(
                    out=o2[:, :b, :],
                    in0=g[:, :b, :D2],
                    in1=rcp[:, :b].unsqueeze(2).broadcast_to([128, b, D2]),
                    op=OP.mult,
                )
                nc.sync.dma_start(
                    out=out[:, :].rearrange("(t p) c -> p t c", p=128)[
                        :, mb : mb + b, :
                    ],
                    in_=o2[:, :b, :],
                )

        # ---------------- program ----------------
        nb0 = (N + 1023) // 1024  # 49
        split_t = CHUNK // 1024  # 24 (straddling block)
        ep = ctx.enter_context(tc.tile_pool(name="ep", bufs=1))
        eps = ctx.enter_context(tc.tile_pool(name="eps", bufs=2))
        epf = ctx.enter_context(tc.tile_pool(name="epf", bufs=4))
        epi = ctx.enter_context(tc.tile_pool(name="epi", bufs=4))

        with tc.tile_pool(name="p0", bufs=3) as p0, tc.tile_pool(
            name="p0ps", bufs=2, space="PSUM"
        ) as p0ps:
            for t in range(split_t + 1):  # chunk 0 (and straddle)
                phase0_block(t, p0, p0ps)
            for t in range(split_t + 1, nb0):  # chunk 1
                phase0_block(t, p0, p0ps)

        aggps = ctx.enter_context(tc.tile_pool(name="aggps", bufs=2, space="PSUM"))
        mp = ctx.enter_context(tc.tile_pool(name="mp", bufs=2))
        mps = ctx.enter_context(tc.tile_pool(name="mps", bufs=2, space="PSUM"))

        zero_table(hn1, 136)  # cols 136:192 are never read
        st10 = edge_prep(1, 0, ep)
        st11 = edge_prep(1, 1, ep)
        edge_batches(1, 0, st10, eps, epf, epi, aggps)
        edge_batches(1, 1, st11, eps, epf, epi, aggps)
        merge1(mp, mps, 0, NT)
        # L2 prep only needs local data -> overlaps the collective
        zero_table(hn2, 128)
        st20 = edge_prep(2, 0, ep)
        st21 = edge_prep(2, 1, ep)
        nc.gpsimd.collective_compute(
            "AllGather",
            OP.bypass,
            ins=[f2cmp[:, :].opt()],
            outs=[f2gath[:, :].opt()],
            replica_groups=[list(range(NCORES))],
        )
        for hh in (0, 1):
            nc.sync.dma_start(
                out=f2tab[hh][:, :D2],
                in_=f2gath[HALF2 * hh : HALF2 * (hh + 1), :],
            )
        edge_batches(2, 0, st20, eps, epf, epi, aggps)
        edge_batches(2, 1, st21, eps, epf, epi, aggps)
        merge2(mp)

    nc.compile()
    return nc


_PROG_CACHE = {}


def _build_cached(sched):
    key = tuple(
        (h, t0, b, K)
        for h in (0, 1)
        for (t0, b, K, off) in sched[h]["batches"]
    )
    if key not in _PROG_CACHE:
        _PROG_CACHE[key] = build(sched)
    return _PROG_CACHE[key]


def _make_in_maps(h, W1, W2, sched, percore):
    hTb = np.ascontiguousarray(np.asarray(h, np.float32).T).astype(bf)
    perm = np.array([(j % 8) * 16 + j // 8 for j in range(128)])
    W1b = np.asarray(W1, np.float32)[:, perm].astype(bf)
    W2b = np.asarray(W2, np.float32)[perm, :].astype(bf)
    ident = np.eye(128, dtype=bf)
    maps = []
    for c in range(NCORES):
        m = {"hTb": hTb, "W1": W1b, "W2b": W2b, "ident": ident}
        for hh in range(2):
            d = percore[c][f"h{hh}"]
            m[f"src1_{hh}"] = wrap16(d["src1"])
            m[f"src2_{hh}"] = wrap16(d["src2"])
            m[f"fd0_{hh}"] = wrap16(d["fd0"])
            m[f"fd1_{hh}"] = wrap16(d["fd1"])
            m[f"fdl_{hh}"] = wrap16(d["fdl"])
            m[f"sct_{hh}"] = wrap16(d["sct"])
            m[f"npad_{hh}"] = d["npad"].astype(np.float32)
        maps.append(m)
    return maps


def kernel(h, W1, W2, src, dst):
    from concourse.bass_utils import run_bass_kernel_spmd

    sched, percore = prep(src, dst)
    nc = _build_cached(sched)
    maps = _make_in_maps(h, W1, W2, sched, percore)
    res = run_bass_kernel_spmd(nc, maps, list(range(NCORES))).results
    out = np.concatenate([res[c]["out"][:NPC] for c in range(NCORES)], axis=0)
    return np.ascontiguousarray(out.astype(np.float32))


# revision 4
# speedup vs baseline: 1.0357x; 1.0357x over previous
"""DotGAT (2-layer dot-product graph attention) on 8 Trainium2 NeuronCores.

V2 design:
- per-(core,half) degree-sorted padded-CSR edge phases (as baseline)
- scatter-add of per-batch segment results into node-id-ordered hn tables
  (merges become plain loads; no pos-gathers, no cross-half add)
- DVE 4D-broadcast attention-weight multiply (no gpsimd exr materialize)
- hybrid fold+reduce aggregation over k
- W2 projection fused into merge1 via PE transpose + matmul (no proj2 phase)
- bf16 [NV_PAD,64] AllGather + pad-expand into gatherable 256B-row tables
- batched f1c writes, bf16 hT loads
"""

import sys

sys.path.insert(0, "/opt/trn_rl_repo")

from contextlib import ExitStack

import numpy as np
import ml_dtypes

import concourse.bass as bass
import concourse.bacc as bacc
import concourse.mybir as mybir
from concourse.tile import TileContext

bf = ml_dtypes.bfloat16


N = 50000
E = 1600000
NCORES = 8
NPC = N // NCORES  # 6250 nodes per core
CHUNK = 25000
SENT = CHUNK  # sentinel row in f1c chunk tables
NT = 49  # tiles of 128 virtual nodes per (core, half)
NV_PAD = NT * 128  # 6272
DUMP = 6256  # dump row (node-id space) for padded scatter rows
SENTL = 6260  # zero row (node-id space) for L2 gathers
HALF2 = NV_PAD * 4  # 25088 rows per half in the f2 tables

dt = mybir.dt
F32, BF16, I16 = dt.float32, dt.bfloat16, dt.int16
AX = mybir.AxisListType
OP = mybir.AluOpType
AF = mybir.ActivationFunctionType

HEADS, HID, D1, D2 = 8, 16, 128, 64
SC1, SC2 = float(HID**-0.5), float(D2**-0.5)
BATCH_CAP = 64  # max slots (b*K) per fs gather batch
MERGE_B = 7  # merge tiles per iteration (49 = 7*7)


def wrap16(idx):
    """int array [S] -> int16 [128, S//16] wrapped+replicated gather layout."""
    S = len(idx)
    assert S % 16 == 0
    w = np.asarray(idx, np.int32).reshape(S // 16, 16).T.astype(np.int16)
    return np.tile(w, (8, 1))


def prep(src, dst):
    """Returns (sched, [per-core data dicts])."""
    src = np.asarray(src, np.int64)
    dst = np.asarray(dst, np.int64)
    core_of = dst // NPC
    half_of = (src >= CHUNK).astype(np.int64)

    order = np.lexsort((dst, half_of, core_of))
    s_src = src[order]
    s_dst = dst[order]
    s_core = core_of[order]
    s_half = half_of[order]

    percore = []
    info = {}
    for c in range(NCORES):
        for h in range(2):
            m = (s_core == c) & (s_half == h)
            esrc = s_src[m]
            edst = s_dst[m] - c * NPC
            deg = np.bincount(edst, minlength=NPC)
            vnodes = np.nonzero(deg > 0)[0]
            nv = len(vnodes)
            assert nv <= NV_PAD, (c, h, nv)
            vorder = vnodes[np.argsort(-deg[vnodes], kind="stable")]
            vdeg = deg[vorder]
            ks = np.zeros(NT, np.int64)
            for t in range(NT):
                seg = vdeg[t * 128 : (t + 1) * 128]
                ks[t] = seg[0] if len(seg) else 1
            ks = np.maximum(ks, 1)
            info[(c, h)] = dict(
                esrc=esrc, edst=edst, deg=deg, vorder=vorder, vdeg=vdeg, ks=ks
            )

    # shared schedule: per half, per tile, max K over cores; batch consecutive
    # tiles into equal-K groups with b*K <= BATCH_CAP and b <= 8.
    sched = {}
    for h in range(2):
        ks_raw = np.max([info[(c, h)]["ks"] for c in range(NCORES)], axis=0)
        batches = []  # (t0, b, K, slot_off)
        t = 0
        off = 0
        ks_pad = np.zeros(NT, np.int64)
        while t < NT:
            K = int(ks_raw[t])
            b = 1
            while (
                t + b < NT
                and b < 4
                and (b + 1) * K <= BATCH_CAP
                and ks_raw[t + b] > K - 6
            ):
                b += 1
            ks_pad[t : t + b] = K
            batches.append((t, b, K, off))
            off += 128 * K * b
            t += b
        sched[h] = {"ks": ks_pad, "batches": batches}

    for c in range(NCORES):
        data = {}
        for h in range(2):
            d = info[(c, h)]
            ks = sched[h]["ks"]
            vorder, vdeg = d["vorder"], d["vdeg"]
            starts = np.zeros(NPC + 1, np.int64)
            np.cumsum(np.bincount(d["edst"], minlength=NPC), out=starts[1:])
            src_slots = []
            npad = np.zeros((128, NT), np.float32)
            fd0 = np.full((NT, 128), SENT, np.int64)
            fd1 = np.full((NT, 128), SENT, np.int64)
            fdl = np.full((NT, 128), SENTL, np.int64)  # L2 fd (local rows)
            sct = np.full((NT, 128), DUMP, np.int64)  # scatter rows (node ids)
            for t in range(NT):
                K = ks[t]
                tile_nodes = vorder[t * 128 : (t + 1) * 128]
                tn = len(tile_nodes)
                slot = np.full((128, K), -1, np.int64)
                for p in range(tn):
                    n = tile_nodes[p]
                    dg = vdeg[t * 128 + p]
                    e0 = starts[n]
                    slot[p, :dg] = d["esrc"][e0 : e0 + dg]
                    npad[p, t] = K - dg
                npad[tn:, t] = K
                gnodes = tile_nodes + c * NPC
                in0 = gnodes < CHUNK
                fd0[t, :tn] = np.where(in0[:tn], gnodes[:tn], SENT)
                fd1[t, :tn] = np.where(~in0[:tn], gnodes[:tn] - CHUNK, SENT)
                fdl[t, :tn] = tile_nodes[:tn]
                sct[t, :tn] = tile_nodes[:tn]
                src_slots.append(slot.T.reshape(-1))  # [K*128] k-major
            allslots = np.concatenate(src_slots)
            pad = allslots < 0
            l1 = np.where(pad, SENT, allslots - CHUNK * h)
            rem = np.where(pad, 0, allslots)
            l2 = NV_PAD * (rem // NPC) + rem % NPC - HALF2 * h
            l2 = np.where(pad, SENTL, l2)
            data[f"h{h}"] = dict(
                src1=l1,
                src2=l2,
                npad=npad,
                fd0=fd0.reshape(-1),
                fd1=fd1.reshape(-1),
                fdl=fdl.reshape(-1),
                sct=sct.reshape(-1),
            )
        percore.append(data)
    return sched, percore


def build(sched):
    nc = bacc.Bacc("TRN2", target_bir_lowering=False, debug=False, num_devices=8)

    S = {h: int(128 * np.sum(sched[h]["ks"])) for h in (0, 1)}

    # ---------------- I/O ----------------
    hTb = nc.declare_dram_parameter("hTb", [128, N], BF16, isOutput=False)
    W1 = nc.declare_dram_parameter("W1", [128, D1], BF16, isOutput=False)
    W2b = nc.declare_dram_parameter("W2b", [128, D2], BF16, isOutput=False)
    ident = nc.declare_dram_parameter("ident", [128, 128], BF16, isOutput=False)
    ins = {}
    for h in (0, 1):
        ins[f"src1_{h}"] = nc.declare_dram_parameter(
            f"src1_{h}", [128, S[h] // 16], I16, isOutput=False
        )
        ins[f"src2_{h}"] = nc.declare_dram_parameter(
            f"src2_{h}", [128, S[h] // 16], I16, isOutput=False
        )
        for nm in ("fd0", "fd1", "fdl", "sct"):
            ins[f"{nm}_{h}"] = nc.declare_dram_parameter(
                f"{nm}_{h}", [128, NT * 8], I16, isOutput=False
            )
        ins[f"npad_{h}"] = nc.declare_dram_parameter(
            f"npad_{h}", [128, NT], F32, isOutput=False
        )
    out = nc.declare_dram_parameter("out", [NV_PAD, D2], F32, isOutput=True)

    with ExitStack() as ctx:
        tc = ctx.enter_context(TileContext(nc))
        dram = ctx.enter_context(tc.tile_pool(name="dram", bufs=1, space="DRAM"))
        f1c = [dram.tile([CHUNK + 1, D1], BF16, tag=f"f1c{i}", name=f"f1c{i}") for i in range(2)]
        hn1 = dram.tile([NV_PAD, 192], F32, name="hn1")
        hn2 = dram.tile([NV_PAD, 128], BF16, name="hn2")
        f2my = dram.tile([NV_PAD, 128], BF16, name="f2my")
        f2cmp = dram.tile([NV_PAD, D2], BF16, name="f2cmp")
        f2gath = dram.tile([NV_PAD * NCORES, D2], BF16, name="f2gath")
        f2tab = [
            dram.tile([HALF2, 128], BF16, name=f"f2tab{i}") for i in range(2)
        ]

        consts = ctx.enter_context(tc.tile_pool(name="consts", bufs=1))
        w1t = consts.tile([128, D1], BF16)
        nc.sync.dma_start(out=w1t[:, :], in_=W1[:, :])
        w2t = consts.tile([128, D2], BF16)
        nc.sync.dma_start(out=w2t[:, :], in_=W2b[:, :])
        idt = consts.tile([128, 128], BF16)
        nc.sync.dma_start(out=idt[:, :], in_=ident[:, :])
        zrow_bf = consts.tile([128, 128], BF16)
        nc.gpsimd.memset(zrow_bf[:, :], 0.0)
        ztile = consts.tile([128, 4, 192], F32)
        nc.gpsimd.memset(ztile[:, :, :], 0.0)
        ztile_bf = consts.tile([128, 4, 128], BF16)
        nc.gpsimd.memset(ztile_bf[:, :, :], 0.0)

        # sentinel zero rows for f1c tables
        nc.sync.dma_start(out=f1c[0][CHUNK : CHUNK + 1, :], in_=zrow_bf[:1, :D1])
        nc.sync.dma_start(out=f1c[1][CHUNK : CHUNK + 1, :], in_=zrow_bf[:1, :D1])

        def zero_table(tab, w, zt):
            for t in range(0, NT, 4):
                b = min(4, NT - t)
                nc.sync.dma_start(
                    out=tab[:, :w].rearrange("(t p) w -> p t w", p=128)[
                        :, t : t + b, :
                    ],
                    in_=zt[:, :b, :w],
                )

        # ------------- Phase 0: feat1 = h @ W1 -> bf16 chunk tables ----------
        def phase0_block(t, p0, p0ps):
            n0 = t * 1024
            bn = min(1024, N - n0)
            lt = p0.tile([128, 1024], BF16, tag="lhst", name="lhst")
            nc.gpsimd.dma_start(out=lt[:, :bn], in_=hTb[:, n0 : n0 + bn])
            ps = p0ps.tile([128, 1024], F32, name="ps0")
            nsub = (bn + 127) // 128
            for s in range(nsub):
                sn = min(128, bn - 128 * s)
                nc.tensor.matmul(
                    ps[:sn, 128 * s : 128 * s + 128],
                    lt[:, 128 * s : 128 * s + sn],
                    w1t[:, :],
                    start=True,
                    stop=True,
                )
            ob = p0.tile([128, 1024], BF16, tag="f1out", name="f1out")
            nc.scalar.activation(ob[:, : 128 * nsub], ps[:, : 128 * nsub], AF.Copy)
            if bn % 128 == 0 and (n0 + bn <= CHUNK or n0 >= CHUNK):
                ci = 0 if n0 < CHUNK else 1
                r0 = n0 - CHUNK * ci
                nc.sync.dma_start(
                    out=f1c[ci][r0 : r0 + bn, :].rearrange(
                        "(s p) c -> p s c", p=128
                    ),
                    in_=ob[:, :bn].rearrange("p (s c) -> p s c", c=128),
                )
            else:
                # straddling block: per-128-row writes with partition splits
                for s in range(nsub):
                    sn = min(128, bn - 128 * s)
                    r0 = n0 + 128 * s
                    obs = ob[:sn, 128 * s : 128 * s + 128]
                    if r0 + sn <= CHUNK:
                        nc.sync.dma_start(out=f1c[0][r0 : r0 + sn, :], in_=obs)
                    elif r0 >= CHUNK:
                        nc.sync.dma_start(
                            out=f1c[1][r0 - CHUNK : r0 - CHUNK + sn, :], in_=obs
                        )
                    else:
                        a = CHUNK - r0
                        nc.sync.dma_start(
                            out=f1c[0][r0 : r0 + a, :],
                            in_=ob[:a, 128 * s : 128 * s + 128],
                        )
                        nc.sync.dma_start(
                            out=f1c[1][0 : sn - a, :],
                            in_=ob[a:sn, 128 * s : 128 * s + 128],
                        )

        # ---------------- edge phase (shared for both layers) ----------------
        def edge_prep(layer, hh, ep):
            # FD tile per half (shared across layers)
            fdt = ep.tile([128, NT, 128], BF16, tag=f"fdt{hh}", name=f"fdt{hh}")
            if layer == 1:
                # fge/fdi shared across halves: freed after the quick add
                fge = ep.tile([128, NT, 128], BF16, tag="fdg", name="fdg")
                fdi = [
                    ep.tile([128, NT * 8], I16, tag=f"fdi_{i}", name=f"fdi_{i}")
                    for i in range(2)
                ]
                for i, fn in enumerate(("fd0", "fd1")):
                    nc.sync.dma_start(out=fdi[i][:, :], in_=ins[f"{fn}_{hh}"][:, :])
                nc.gpsimd.dma_gather(
                    out_ap=fdt[:, :, :],
                    in_ap=f1c[0][:, :],
                    idxs_ap=fdi[0][:, :],
                    num_idxs=NT * 128,
                    num_idxs_reg=NT * 128,
                    elem_size=128,
                    single_packet=False,
                )
                nc.gpsimd.dma_gather(
                    out_ap=fge[:, :, :],
                    in_ap=f1c[1][:, :],
                    idxs_ap=fdi[1][:, :],
                    num_idxs=NT * 128,
                    num_idxs_reg=NT * 128,
                    elem_size=128,
                    single_packet=False,
                )
                nc.vector.tensor_tensor(
                    out=fdt[:, :, :], in0=fdt[:, :, :], in1=fge[:, :, :], op=OP.add
                )
            else:
                fdi = ep.tile([128, NT * 8], I16, tag="fdi_0", name="fdi_0")
                nc.sync.dma_start(out=fdi[:, :], in_=ins[f"fdl_{hh}"][:, :])
                nc.gpsimd.dma_gather(
                    out_ap=fdt[:, :, :],
                    in_ap=f2my[:, :],
                    idxs_ap=fdi[:, :],
                    num_idxs=NT * 128,
                    num_idxs_reg=NT * 128,
                    elem_size=128,
                    single_packet=False,
                )
            npt = ep.tile([128, NT], F32, tag=f"npad{hh}", name=f"npad{hh}")
            nc.sync.dma_start(out=npt[:, :], in_=ins[f"npad_{hh}"][:, :])
            sci = ep.tile([128, NT * 8], I16, tag=f"sct{hh}", name=f"sct{hh}")
            nc.sync.dma_start(out=sci[:, :], in_=ins[f"sct_{hh}"][:, :])
            itall = None
            if layer == 2 and hh == 0:
                itall = ep.tile(
                    [128, S[hh] // 16], I16, tag=f"itall{hh}", name=f"itall{hh}"
                )
                nc.sync.dma_start(out=itall[:, :], in_=ins[f"src2_{hh}"][:, :])
            return fdt, npt, sci, itall

        def edge_batches(layer, hh, state, eps, epf, epi, aggps):
            fdt, npt, sci, itall = state
            D = D1 if layer == 1 else D2
            H = HEADS if layer == 1 else 1
            C = D // H
            scale = SC1 if layer == 1 else SC2
            srcname = f"src{layer}_{hh}"
            hntab = hn1 if layer == 1 else hn2
            stg_w = 192 if layer == 1 else 128
            den0 = D  # denominator column offset within stg
            batches = sched[hh]["batches"]

            ftab = f1c[hh][:, :] if layer == 1 else f2tab[hh][:, :]

            # software-pipelined batch loop (3 skewed stages):
            #   stage0(i): idx load + fs gather          [SP/Pool/DMA]
            #   stage1(i): prod, fold, exp               [DVE, Act]
            #   stage2(i): den, wa, agg, scatter         [DVE, Pool]
            # Issue order per iteration: s0(i+1), s1(i), s2(i-1) so the DVE
            # never head-of-line blocks on a gather or on the Act exp.
            live = {}

            def s0(i):
                t0, b, K, off = batches[i]
                bK = b * K
                if itall is not None:
                    it = itall[:, off // 16 : off // 16 + 8 * bK]
                else:
                    itt = epi.tile([128, 8 * BATCH_CAP], I16, tag="srcidx")
                    nc.sync.dma_start(
                        out=itt[:, : 8 * bK],
                        in_=ins[srcname][:, off // 16 : off // 16 + 8 * bK],
                    )
                    it = itt[:, : 8 * bK]
                fsg = epf.tile([128, BATCH_CAP, 128], BF16, tag="fsg")
                nc.gpsimd.dma_gather(
                    out_ap=fsg[:, :bK, :],
                    in_ap=ftab,
                    idxs_ap=it,
                    num_idxs=128 * bK,
                    num_idxs_reg=128 * bK,
                    elem_size=128,
                    single_packet=False,
                )
                live[i] = {"fsg": fsg}

            def s1(i):
                t0, b, K, off = batches[i]
                bK = b * K
                fsg = live[i]["fsg"]
                prod = eps.tile([128, BATCH_CAP, D1], BF16, tag="pw")
                for j in range(b):
                    nc.vector.tensor_tensor(
                        out=prod[:, j * K : (j + 1) * K, :D],
                        in0=fsg[:, j * K : (j + 1) * K, :D],
                        in1=fdt[:, t0 + j, :D].unsqueeze(1).broadcast_to([128, K, D]),
                        op=OP.mult,
                    )
                cur = D
                while cur > H:
                    half = cur // 2
                    nc.vector.tensor_tensor(
                        out=prod[:, :bK, :half],
                        in0=prod[:, :bK, :half],
                        in1=prod[:, :bK, half:cur],
                        op=OP.add,
                    )
                    cur = half
                ex = eps.tile([128, BATCH_CAP * HEADS], BF16, tag="ex")
                exv = ex[:, : bK * H].rearrange("p (k h) -> p k h", k=bK)
                nc.scalar.activation(exv, prod[:, :bK, :H], AF.Exp, scale=scale)
                live[i]["ex"] = ex

            def s2(i):
                t0, b, K, off = batches[i]
                bK = b * K
                fsg = live[i]["fsg"]
                ex = live[i]["ex"]
                if layer == 1:
                    stg = eps.tile([128, 4 * 192], F32, tag="stg")
                else:
                    stg = eps.tile([128, 4 * 128], BF16, tag="stg2")
                stg4 = stg[:, : b * stg_w].rearrange("p (b w) -> p b w", b=b)
                if i < 2:
                    # zero the pad columns once per rotating buffer; later
                    # batches never write them so they stay zero
                    nc.gpsimd.memset(
                        stg[:, :].rearrange("p (b w) -> p b w", w=stg_w)[
                            :, :, den0 + H :
                        ],
                        0.0,
                    )
                with nc.allow_low_precision(reason="bf16 hn2 staging"):
                    nc.vector.tensor_reduce(
                        out=stg4[:, :, den0 : den0 + H],
                        in_=ex[:, : bK * H].rearrange(
                            "p (b k h) -> p b h k", b=b, k=K
                        ),
                        axis=AX.X,
                        op=OP.add,
                    )
                nc.vector.tensor_tensor(
                    out=stg4[:, :, den0 : den0 + H],
                    in0=stg4[:, :, den0 : den0 + H],
                    in1=npt[:, t0 : t0 + b].unsqueeze(2).broadcast_to([128, b, H]),
                    op=OP.subtract,
                )
                # weighted features in-place: fsg *= ex (broadcast over c).
                # Split ~1/4 of the slots onto the Pool engine to offload DVE.
                if H > 1:
                    kp = (bK // 4) & ~1
                    if kp >= 8:
                        nc.gpsimd.tensor_tensor(
                            out=fsg[:, :kp, :D].rearrange(
                                "p k (c h) -> p k c h", c=C
                            ),
                            in0=fsg[:, :kp, :D].rearrange(
                                "p k (c h) -> p k c h", c=C
                            ),
                            in1=ex[:, : kp * H]
                            .rearrange("p (k h) -> p k h", k=kp)
                            .unsqueeze(2)
                            .broadcast_to([128, kp, C, H]),
                            op=OP.mult,
                        )
                    else:
                        kp = 0
                    nc.vector.tensor_tensor(
                        out=fsg[:, kp:bK, :D].rearrange(
                            "p k (c h) -> p k c h", c=C
                        ),
                        in0=fsg[:, kp:bK, :D].rearrange(
                            "p k (c h) -> p k c h", c=C
                        ),
                        in1=ex[:, kp * H : bK * H]
                        .rearrange("p (k h) -> p k h", k=bK - kp)
                        .unsqueeze(2)
                        .broadcast_to([128, bK - kp, C, H]),
                        op=OP.mult,
                    )
                else:
                    nc.vector.tensor_tensor(
                        out=fsg[:, :bK, :D],
                        in0=fsg[:, :bK, :D],
                        in1=ex[:, :bK].unsqueeze(2).broadcast_to([128, bK, D]),
                        op=OP.mult,
                    )
                if layer == 1:
                    # aggregation over k on the PE: transpose-accumulate each
                    # weighted slot into PSUM (f32), then transpose back.
                    aggT = aggps.tile([128, 512], F32, name="aggT")
                    for j in range(b):
                        for k in range(K):
                            nc.tensor.matmul(
                                aggT[:, j * 128 : (j + 1) * 128],
                                fsg[:, j * K + k, :],
                                idt[:, :],
                                start=(k == 0),
                                stop=(k == K - 1),
                            )
                    aggTs = eps.tile([128, 512], BF16, tag="aggTs")
                    nc.scalar.activation(
                        aggTs[:, : b * 128], aggT[:, : b * 128], AF.Copy
                    )
                    nodeT = aggps.tile([128, 512], F32, name="nodeT")
                    for j in range(b):
                        nc.tensor.matmul(
                            nodeT[:, j * 128 : (j + 1) * 128],
                            aggTs[:, j * 128 : (j + 1) * 128],
                            idt[:, :],
                            start=True,
                            stop=True,
                        )
                    nc.scalar.activation(
                        stg4[:, :, :D],
                        nodeT[:, : b * 128].rearrange("p (b d) -> p b d", b=b),
                        AF.Copy,
                    )
                else:
                    # aggregation over k: fold (bf16) then reduce (f32 accum)
                    v = fsg[:, :bK, :D].rearrange("p (b k) d -> p b k d", b=b)
                    cur = K
                    folds = 0
                    while folds < 2 and cur > 4:
                        if cur % 2:
                            nc.vector.tensor_tensor(
                                out=v[:, :, 0:1, :D],
                                in0=v[:, :, 0:1, :D],
                                in1=v[:, :, cur - 1 : cur, :D],
                                op=OP.add,
                            )
                            cur -= 1
                        h2 = cur // 2
                        nc.vector.tensor_tensor(
                            out=v[:, :, :h2, :D],
                            in0=v[:, :, :h2, :D],
                            in1=v[:, :, h2:cur, :D],
                            op=OP.add,
                        )
                        cur = h2
                        folds += 1
                    with nc.allow_low_precision(reason="bf16 hn2 staging"):
                        nc.vector.tensor_reduce(
                            out=stg4[:, :, :D],
                            in_=v[:, :, :cur, :D].rearrange("p b k d -> p b d k"),
                            axis=AX.X,
                            op=OP.add,
                        )
                nc.gpsimd.dma_scatter_add(
                    out_ap=hntab[:, :stg_w],
                    in_ap=stg4[:, :, :],
                    idxs_ap=sci[:, 8 * t0 : 8 * (t0 + b)],
                    num_idxs=128 * b,
                    num_idxs_reg=128 * b,
                    elem_size=stg_w,
                    single_packet=False,
                )
                del live[i]

            nbat = len(batches)
            s0(0)
            if nbat > 1:
                s0(1)
            for i in range(nbat):
                if i + 2 < nbat:
                    s0(i + 2)
                s1(i)
                if i >= 1:
                    s2(i - 1)
            s2(nbat - 1)

        # ---------------- merge1 + local W2 projection ----------------
        def merge1(mp, mps, mb_lo, mb_hi):
            for mb in range(mb_lo, mb_hi, MERGE_B):
                b = min(MERGE_B, NT - mb)
                g = mp.tile([128, MERGE_B, 136], F32, tag="g1", name="g1")
                nc.sync.dma_start(
                    out=g[:, :b, :],
                    in_=hn1[:, :136].rearrange("(t p) w -> p t w", p=128)[
                        :, mb : mb + b, :
                    ],
                )
                den = mp.tile([128, MERGE_B * 8], F32, tag="den")
                nc.vector.tensor_scalar_max(den[:, : b * 8], g[:, :b, 128:136], 1e-9)
                rcp = mp.tile([128, MERGE_B * 8], F32, tag="rcp")
                nc.vector.reciprocal(rcp[:, : b * 8], den[:, : b * 8])
                o1 = mp.tile([128, MERGE_B, 128], F32, tag="o1")
                nc.vector.tensor_tensor(
                    out=o1[:, :b, :].rearrange("p b (c h) -> p b c h", c=HID),
                    in0=g[:, :b, :128].rearrange("p b (c h) -> p b c h", c=HID),
                    in1=rcp[:, : b * 8]
                    .rearrange("p (b h) -> p b h", b=b)
                    .unsqueeze(2)
                    .broadcast_to([128, b, HID, HEADS]),
                    op=OP.mult,
                )
                # ELU -> bf16 (2-byte ops get 2x DVE throughput)
                mx = mp.tile([128, MERGE_B, 128], BF16, tag="mx")
                nc.vector.tensor_scalar_max(mx[:, :b, :], o1[:, :b, :], 0.0)
                mn = mp.tile([128, MERGE_B, 128], BF16, tag="mn")
                nc.vector.tensor_scalar_min(mn[:, :b, :], o1[:, :b, :], 0.0)
                e = mp.tile([128, MERGE_B, 128], BF16, tag="e")
                nc.scalar.activation(e[:, :b, :], mn[:, :b, :], AF.Exp)
                nc.vector.tensor_scalar_add(e[:, :b, :], e[:, :b, :], -1.0)
                h1t = mp.tile([128, MERGE_B, 128], BF16, tag="h1t")
                nc.vector.tensor_tensor(
                    out=h1t[:, :b, :], in0=mx[:, :b, :], in1=e[:, :b, :], op=OP.add
                )
                # fused projection: f2 = h1 @ W2 per 128-node tile
                f2p = mp.tile([128, MERGE_B, 128], BF16, tag="f2p")
                nc.gpsimd.memset(f2p[:, :b, D2:], 0.0)
                for j in range(b):
                    psT = mps.tile([128, 128], BF16, name="psT")
                    nc.tensor.matmul(
                        psT[:, :], h1t[:, j, :], idt[:, :],
                        start=True, stop=True, is_transpose=True,
                    )
                    h1T = mp.tile([128, 128], BF16, tag="h1T")
                    nc.scalar.activation(h1T[:, :], psT[:, :], AF.Copy)
                    ps2 = mps.tile([128, D2], F32, name="ps2")
                    nc.tensor.matmul(
                        ps2[:, :], h1T[:, :], w2t[:, :], start=True, stop=True
                    )
                    nc.scalar.activation(f2p[:, j, :D2], ps2[:, :], AF.Copy)
                nc.sync.dma_start(
                    out=f2my[:, :].rearrange("(t p) c -> p t c", p=128)[
                        :, mb : mb + b, :
                    ],
                    in_=f2p[:, :b, :],
                )
                nc.sync.dma_start(
                    out=f2cmp[:, :].rearrange("(t p) c -> p t c", p=128)[
                        :, mb : mb + b, :
                    ],
                    in_=f2p[:, :b, :D2],
                )

        # ---------------- merge2 -> output ----------------
        def merge2(mp):
            for mb in range(0, NT, MERGE_B):
                b = min(MERGE_B, NT - mb)
                g = mp.tile([128, MERGE_B, 65], BF16, tag="g2", name="g2")
                nc.sync.dma_start(
                    out=g[:, :b, :],
                    in_=hn2[:, :65].rearrange("(t p) w -> p t w", p=128)[
                        :, mb : mb + b, :
                    ],
                )
                den = mp.tile([128, MERGE_B], F32, tag="den2")
                nc.vector.tensor_scalar_max(den[:, :b], g[:, :b, 64:65], 1e-9)
                rcp = mp.tile([128, MERGE_B], F32, tag="rcp2")
                nc.vector.reciprocal(rcp[:, :b], den[:, :b])
                o2 = mp.tile([128, MERGE_B, D2], F32, tag="o2")
                nc.vector.tensor_tensor(
                    out=o2[:, :b, :],
                    in0=g[:, :b, :D2],
                    in1=rcp[:, :b].unsqueeze(2).broadcast_to([128, b, D2]),
                    op=OP.mult,
                )
                nc.sync.dma_start(
                    out=out[:, :].rearrange("(t p) c -> p t c", p=128)[
                        :, mb : mb + b, :
                    ],
                    in_=o2[:, :b, :],
                )

        # ---------------- program ----------------
        nb0 = (N + 1023) // 1024  # 49
        split_t = CHUNK // 1024  # 24 (straddling block)
        ep = ctx.enter_context(tc.tile_pool(name="ep", bufs=1))
        eps = ctx.enter_context(tc.tile_pool(name="eps", bufs=2))
        epf = ctx.enter_context(tc.tile_pool(name="epf", bufs=4))
        epi = ctx.enter_context(tc.tile_pool(name="epi", bufs=3))

        with tc.tile_pool(name="p0", bufs=3) as p0, tc.tile_pool(
            name="p0ps", bufs=2, space="PSUM"
        ) as p0ps:
            for t in range(split_t + 1):  # chunk 0 (and straddle)
                phase0_block(t, p0, p0ps)
            for t in range(split_t + 1, nb0):  # chunk 1
                phase0_block(t, p0, p0ps)

        aggps = ctx.enter_context(tc.tile_pool(name="aggps", bufs=2, space="PSUM"))
        mp = ctx.enter_context(tc.tile_pool(name="mp", bufs=2))
        mps = ctx.enter_context(tc.tile_pool(name="mps", bufs=2, space="PSUM"))

        zero_table(hn1, 136, ztile)  # cols 136:192 are never read
        st10 = edge_prep(1, 0, ep)
        st11 = edge_prep(1, 1, ep)
        edge_batches(1, 0, st10, eps, epf, epi, aggps)
        edge_batches(1, 1, st11, eps, epf, epi, aggps)
        merge1(mp, mps, 0, NT)
        # L2 prep only needs local data -> overlaps the collective
        zero_table(hn2, 128, ztile_bf)
        st20 = edge_prep(2, 0, ep)
        st21 = edge_prep(2, 1, ep)
        nc.gpsimd.collective_compute(
            "AllGather",
            OP.bypass,
            ins=[f2cmp[:, :].opt()],
            outs=[f2gath[:, :].opt()],
            replica_groups=[list(range(NCORES))],
        )
        for hh in (0, 1):
            nc.sync.dma_start(
                out=f2tab[hh][:, :D2],
                in_=f2gath[HALF2 * hh : HALF2 * (hh + 1), :],
            )
        edge_batches(2, 0, st20, eps, epf, epi, aggps)
        edge_batches(2, 1, st21, eps, epf, epi, aggps)
        merge2(mp)

    nc.compile()
    return nc


_PROG_CACHE = {}


def _build_cached(sched):
    key = tuple(
        (h, t0, b, K)
        for h in (0, 1)
        for (t0, b, K, off) in sched[h]["batches"]
    )
    if key not in _PROG_CACHE:
        _PROG_CACHE[key] = build(sched)
    return _PROG_CACHE[key]


def _make_in_maps(h, W1, W2, sched, percore):
    hTb = np.ascontiguousarray(np.asarray(h, np.float32).T).astype(bf)
    perm = np.array([(j % 8) * 16 + j // 8 for j in range(128)])
    W1b = np.asarray(W1, np.float32)[:, perm].astype(bf)
    W2b = np.asarray(W2, np.float32)[perm, :].astype(bf)
    ident = np.eye(128, dtype=bf)
    maps = []
    for c in range(NCORES):
        m = {"hTb": hTb, "W1": W1b, "W2b": W2b, "ident": ident}
        for hh in range(2):
            d = percore[c][f"h{hh}"]
            m[f"src1_{hh}"] = wrap16(d["src1"])
            m[f"src2_{hh}"] = wrap16(d["src2"])
            m[f"fd0_{hh}"] = wrap16(d["fd0"])
            m[f"fd1_{hh}"] = wrap16(d["fd1"])
            m[f"fdl_{hh}"] = wrap16(d["fdl"])
            m[f"sct_{hh}"] = wrap16(d["sct"])
            m[f"npad_{hh}"] = d["npad"].astype(np.float32)
        maps.append(m)
    return maps


def kernel(h, W1, W2, src, dst):
    from concourse.bass_utils import run_bass_kernel_spmd

    sched, percore = prep(src, dst)
    nc = _build_cached(sched)
    maps = _make_in_maps(h, W1, W2, sched, percore)
    res = run_bass_kernel_spmd(nc, maps, list(range(NCORES))).results
    out = np.concatenate([res[c]["out"][:NPC] for c in range(NCORES)], axis=0)
    return np.ascontiguousarray(out.astype(np.float32))


# revision 5
# speedup vs baseline: 1.0447x; 1.0087x over previous
"""DotGAT (2-layer dot-product graph attention) on 8 Trainium2 NeuronCores.

V2 design:
- per-(core,half) degree-sorted padded-CSR edge phases (as baseline)
- scatter-add of per-batch segment results into node-id-ordered hn tables
  (merges become plain loads; no pos-gathers, no cross-half add)
- DVE 4D-broadcast attention-weight multiply (no gpsimd exr materialize)
- hybrid fold+reduce aggregation over k
- W2 projection fused into merge1 via PE transpose + matmul (no proj2 phase)
- bf16 [NV_PAD,64] AllGather + pad-expand into gatherable 256B-row tables
- batched f1c writes, bf16 hT loads
"""

import sys

sys.path.insert(0, "/opt/trn_rl_repo")

from contextlib import ExitStack

import numpy as np
import ml_dtypes

import concourse.bass as bass
import concourse.bacc as bacc
import concourse.mybir as mybir
from concourse.tile import TileContext

bf = ml_dtypes.bfloat16


N = 50000
E = 1600000
NCORES = 8
NPC = N // NCORES  # 6250 nodes per core
CHUNK = 25000
SENT = CHUNK  # sentinel row in f1c chunk tables
NT = 49  # tiles of 128 virtual nodes per (core, half)
NV_PAD = NT * 128  # 6272
DUMP = 6256  # dump row (node-id space) for padded scatter rows
SENTL = 6260  # zero row (node-id space) for L2 gathers
HALF2 = NV_PAD * 4  # 25088 rows per half in the f2 tables

dt = mybir.dt
F32, BF16, I16 = dt.float32, dt.bfloat16, dt.int16
AX = mybir.AxisListType
OP = mybir.AluOpType
AF = mybir.ActivationFunctionType

HEADS, HID, D1, D2 = 8, 16, 128, 64
SC1, SC2 = float(HID**-0.5), float(D2**-0.5)
BATCH_CAP = 64  # max slots (b*K) per fs gather batch
MERGE_B = 7  # merge tiles per iteration (49 = 7*7)


def wrap16(idx):
    """int array [S] -> int16 [128, S//16] wrapped+replicated gather layout."""
    S = len(idx)
    assert S % 16 == 0
    w = np.asarray(idx, np.int32).reshape(S // 16, 16).T.astype(np.int16)
    return np.tile(w, (8, 1))


def prep(src, dst):
    """Returns (sched, [per-core data dicts])."""
    src = np.asarray(src, np.int64)
    dst = np.asarray(dst, np.int64)
    core_of = dst // NPC
    half_of = (src >= CHUNK).astype(np.int64)

    order = np.lexsort((dst, half_of, core_of))
    s_src = src[order]
    s_dst = dst[order]
    s_core = core_of[order]
    s_half = half_of[order]

    percore = []
    info = {}
    for c in range(NCORES):
        for h in range(2):
            m = (s_core == c) & (s_half == h)
            esrc = s_src[m]
            edst = s_dst[m] - c * NPC
            deg = np.bincount(edst, minlength=NPC)
            vnodes = np.nonzero(deg > 0)[0]
            nv = len(vnodes)
            assert nv <= NV_PAD, (c, h, nv)
            vorder = vnodes[np.argsort(-deg[vnodes], kind="stable")]
            vdeg = deg[vorder]
            ks = np.zeros(NT, np.int64)
            for t in range(NT):
                seg = vdeg[t * 128 : (t + 1) * 128]
                ks[t] = seg[0] if len(seg) else 1
            ks = np.maximum(ks, 1)
            info[(c, h)] = dict(
                esrc=esrc, edst=edst, deg=deg, vorder=vorder, vdeg=vdeg, ks=ks
            )

    # shared schedule: per half, per tile, max K over cores; batch consecutive
    # tiles into equal-K groups with b*K <= BATCH_CAP and b <= 8.
    sched = {}
    for h in range(2):
        ks_raw = np.max([info[(c, h)]["ks"] for c in range(NCORES)], axis=0)
        batches = []  # (t0, b, K, slot_off)
        t = 0
        off = 0
        ks_pad = np.zeros(NT, np.int64)
        while t < NT:
            K = int(ks_raw[t])
            b = 1
            while (
                t + b < NT
                and b < 4
                and (b + 1) * K <= BATCH_CAP
                and ks_raw[t + b] > K - 6
            ):
                b += 1
            ks_pad[t : t + b] = K
            batches.append((t, b, K, off))
            off += 128 * K * b
            t += b
        sched[h] = {"ks": ks_pad, "batches": batches}

    for c in range(NCORES):
        data = {}
        for h in range(2):
            d = info[(c, h)]
            ks = sched[h]["ks"]
            vorder, vdeg = d["vorder"], d["vdeg"]
            starts = np.zeros(NPC + 1, np.int64)
            np.cumsum(np.bincount(d["edst"], minlength=NPC), out=starts[1:])
            src_slots = []
            npad = np.zeros((128, NT), np.float32)
            fd0 = np.full((NT, 128), SENT, np.int64)
            fd1 = np.full((NT, 128), SENT, np.int64)
            fdl = np.full((NT, 128), SENTL, np.int64)  # L2 fd (local rows)
            sct = np.full((NT, 128), DUMP, np.int64)  # scatter rows (node ids)
            for t in range(NT):
                K = ks[t]
                tile_nodes = vorder[t * 128 : (t + 1) * 128]
                tn = len(tile_nodes)
                slot = np.full((128, K), -1, np.int64)
                for p in range(tn):
                    n = tile_nodes[p]
                    dg = vdeg[t * 128 + p]
                    e0 = starts[n]
                    slot[p, :dg] = d["esrc"][e0 : e0 + dg]
                    npad[p, t] = K - dg
                npad[tn:, t] = K
                gnodes = tile_nodes + c * NPC
                in0 = gnodes < CHUNK
                fd0[t, :tn] = np.where(in0[:tn], gnodes[:tn], SENT)
                fd1[t, :tn] = np.where(~in0[:tn], gnodes[:tn] - CHUNK, SENT)
                fdl[t, :tn] = tile_nodes[:tn]
                sct[t, :tn] = tile_nodes[:tn]
                src_slots.append(slot.T.reshape(-1))  # [K*128] k-major
            allslots = np.concatenate(src_slots)
            pad = allslots < 0
            l1 = np.where(pad, SENT, allslots - CHUNK * h)
            rem = np.where(pad, 0, allslots)
            l2 = NV_PAD * (rem // NPC) + rem % NPC - HALF2 * h
            l2 = np.where(pad, SENTL, l2)
            data[f"h{h}"] = dict(
                src1=l1,
                src2=l2,
                npad=npad,
                fd0=fd0.reshape(-1),
                fd1=fd1.reshape(-1),
                fdl=fdl.reshape(-1),
                sct=sct.reshape(-1),
            )
        percore.append(data)
    return sched, percore


def build(sched):
    nc = bacc.Bacc("TRN2", target_bir_lowering=False, debug=False, num_devices=8)

    S = {h: int(128 * np.sum(sched[h]["ks"])) for h in (0, 1)}

    # ---------------- I/O ----------------
    hTb = nc.declare_dram_parameter("hTb", [128, N], BF16, isOutput=False)
    W1 = nc.declare_dram_parameter("W1", [128, D1], BF16, isOutput=False)
    W2b = nc.declare_dram_parameter("W2b", [128, D2], BF16, isOutput=False)
    ident = nc.declare_dram_parameter("ident", [128, 128], BF16, isOutput=False)
    ins = {}
    for h in (0, 1):
        ins[f"src1_{h}"] = nc.declare_dram_parameter(
            f"src1_{h}", [128, S[h] // 16], I16, isOutput=False
        )
        ins[f"src2_{h}"] = nc.declare_dram_parameter(
            f"src2_{h}", [128, S[h] // 16], I16, isOutput=False
        )
        for nm in ("fd0", "fd1", "fdl", "sct"):
            ins[f"{nm}_{h}"] = nc.declare_dram_parameter(
                f"{nm}_{h}", [128, NT * 8], I16, isOutput=False
            )
        ins[f"npad_{h}"] = nc.declare_dram_parameter(
            f"npad_{h}", [128, NT], F32, isOutput=False
        )
    out = nc.declare_dram_parameter("out", [NV_PAD, D2], F32, isOutput=True)

    with ExitStack() as ctx:
        tc = ctx.enter_context(TileContext(nc))
        dram = ctx.enter_context(tc.tile_pool(name="dram", bufs=1, space="DRAM"))
        f1c = [dram.tile([CHUNK + 1, D1], BF16, tag=f"f1c{i}", name=f"f1c{i}") for i in range(2)]
        hn1 = dram.tile([NV_PAD, 192], F32, name="hn1")
        hn2 = dram.tile([NV_PAD, 128], BF16, name="hn2")
        f2my = dram.tile([NV_PAD, 128], BF16, name="f2my")
        f2cmp = dram.tile([NV_PAD, D2], BF16, name="f2cmp")
        f2gath = dram.tile([NV_PAD * NCORES, D2], BF16, name="f2gath")
        f2tab = [
            dram.tile([HALF2, 128], BF16, name=f"f2tab{i}") for i in range(2)
        ]

        consts = ctx.enter_context(tc.tile_pool(name="consts", bufs=1))
        w1t = consts.tile([128, D1], BF16)
        nc.sync.dma_start(out=w1t[:, :], in_=W1[:, :])
        w2t = consts.tile([128, D2], BF16)
        nc.sync.dma_start(out=w2t[:, :], in_=W2b[:, :])
        idt = consts.tile([128, 128], BF16)
        nc.sync.dma_start(out=idt[:, :], in_=ident[:, :])
        zrow_bf = consts.tile([128, 128], BF16)
        nc.gpsimd.memset(zrow_bf[:, :], 0.0)
        ztile = consts.tile([128, 4, 192], F32)
        nc.gpsimd.memset(ztile[:, :, :], 0.0)
        ztile_bf = consts.tile([128, 4, 128], BF16)
        nc.gpsimd.memset(ztile_bf[:, :, :], 0.0)

        # sentinel zero rows for f1c tables
        nc.sync.dma_start(out=f1c[0][CHUNK : CHUNK + 1, :], in_=zrow_bf[:1, :D1])
        nc.sync.dma_start(out=f1c[1][CHUNK : CHUNK + 1, :], in_=zrow_bf[:1, :D1])

        def zero_table(tab, w, zt):
            for t in range(0, NT, 4):
                b = min(4, NT - t)
                nc.sync.dma_start(
                    out=tab[:, :w].rearrange("(t p) w -> p t w", p=128)[
                        :, t : t + b, :
                    ],
                    in_=zt[:, :b, :w],
                )

        # ------------- Phase 0: feat1 = h @ W1 -> bf16 chunk tables ----------
        def phase0_block(t, p0, p0ps):
            n0 = t * 1024
            bn = min(1024, N - n0)
            lt = p0.tile([128, 1024], BF16, tag="lhst", name="lhst")
            nc.gpsimd.dma_start(out=lt[:, :bn], in_=hTb[:, n0 : n0 + bn])
            ps = p0ps.tile([128, 1024], F32, name="ps0")
            nsub = (bn + 127) // 128
            for s in range(nsub):
                sn = min(128, bn - 128 * s)
                nc.tensor.matmul(
                    ps[:sn, 128 * s : 128 * s + 128],
                    lt[:, 128 * s : 128 * s + sn],
                    w1t[:, :],
                    start=True,
                    stop=True,
                )
            ob = p0.tile([128, 1024], BF16, tag="f1out", name="f1out")
            nc.scalar.activation(ob[:, : 128 * nsub], ps[:, : 128 * nsub], AF.Copy)
            if bn % 128 == 0 and (n0 + bn <= CHUNK or n0 >= CHUNK):
                ci = 0 if n0 < CHUNK else 1
                r0 = n0 - CHUNK * ci
                nc.sync.dma_start(
                    out=f1c[ci][r0 : r0 + bn, :].rearrange(
                        "(s p) c -> p s c", p=128
                    ),
                    in_=ob[:, :bn].rearrange("p (s c) -> p s c", c=128),
                )
            else:
                # straddling block: per-128-row writes with partition splits
                for s in range(nsub):
                    sn = min(128, bn - 128 * s)
                    r0 = n0 + 128 * s
                    obs = ob[:sn, 128 * s : 128 * s + 128]
                    if r0 + sn <= CHUNK:
                        nc.sync.dma_start(out=f1c[0][r0 : r0 + sn, :], in_=obs)
                    elif r0 >= CHUNK:
                        nc.sync.dma_start(
                            out=f1c[1][r0 - CHUNK : r0 - CHUNK + sn, :], in_=obs
                        )
                    else:
                        a = CHUNK - r0
                        nc.sync.dma_start(
                            out=f1c[0][r0 : r0 + a, :],
                            in_=ob[:a, 128 * s : 128 * s + 128],
                        )
                        nc.sync.dma_start(
                            out=f1c[1][0 : sn - a, :],
                            in_=ob[a:sn, 128 * s : 128 * s + 128],
                        )

        # ---------------- edge phase (shared for both layers) ----------------
        def edge_prep(layer, hh, ep):
            # FD tile per half (shared across layers)
            fdt = ep.tile([128, NT, 128], BF16, tag=f"fdt{hh}", name=f"fdt{hh}")
            if layer == 1:
                # fge/fdi shared across halves: freed after the quick add
                fge = ep.tile([128, NT, 128], BF16, tag="fdg", name="fdg")
                fdi = [
                    ep.tile([128, NT * 8], I16, tag=f"fdi_{i}", name=f"fdi_{i}")
                    for i in range(2)
                ]
                for i, fn in enumerate(("fd0", "fd1")):
                    nc.sync.dma_start(out=fdi[i][:, :], in_=ins[f"{fn}_{hh}"][:, :])
                nc.gpsimd.dma_gather(
                    out_ap=fdt[:, :, :],
                    in_ap=f1c[0][:, :],
                    idxs_ap=fdi[0][:, :],
                    num_idxs=NT * 128,
                    num_idxs_reg=NT * 128,
                    elem_size=128,
                    single_packet=False,
                )
                nc.gpsimd.dma_gather(
                    out_ap=fge[:, :, :],
                    in_ap=f1c[1][:, :],
                    idxs_ap=fdi[1][:, :],
                    num_idxs=NT * 128,
                    num_idxs_reg=NT * 128,
                    elem_size=128,
                    single_packet=False,
                )
                nc.vector.tensor_tensor(
                    out=fdt[:, :, :], in0=fdt[:, :, :], in1=fge[:, :, :], op=OP.add
                )
            else:
                fdi = ep.tile([128, NT * 8], I16, tag="fdi_0", name="fdi_0")
                nc.sync.dma_start(out=fdi[:, :], in_=ins[f"fdl_{hh}"][:, :])
                nc.gpsimd.dma_gather(
                    out_ap=fdt[:, :, :],
                    in_ap=f2my[:, :],
                    idxs_ap=fdi[:, :],
                    num_idxs=NT * 128,
                    num_idxs_reg=NT * 128,
                    elem_size=128,
                    single_packet=False,
                )
            npt = ep.tile([128, NT], F32, tag=f"npad{hh}", name=f"npad{hh}")
            nc.sync.dma_start(out=npt[:, :], in_=ins[f"npad_{hh}"][:, :])
            sci = ep.tile([128, NT * 8], I16, tag=f"sct{hh}", name=f"sct{hh}")
            nc.sync.dma_start(out=sci[:, :], in_=ins[f"sct_{hh}"][:, :])
            itall = None
            if layer == 2 and hh == 0:
                itall = ep.tile(
                    [128, S[hh] // 16], I16, tag=f"itall{hh}", name=f"itall{hh}"
                )
                nc.sync.dma_start(out=itall[:, :], in_=ins[f"src2_{hh}"][:, :])
            return fdt, npt, sci, itall

        def edge_batches(layer, hh, state, eps, epf, epi, aggps):
            fdt, npt, sci, itall = state
            D = D1 if layer == 1 else D2
            H = HEADS if layer == 1 else 1
            C = D // H
            scale = SC1 if layer == 1 else SC2
            srcname = f"src{layer}_{hh}"
            hntab = hn1 if layer == 1 else hn2
            stg_w = 192 if layer == 1 else 128
            den0 = D  # denominator column offset within stg
            batches = sched[hh]["batches"]

            ftab = f1c[hh][:, :] if layer == 1 else f2tab[hh][:, :]

            # software-pipelined batch loop (3 skewed stages):
            #   stage0(i): idx load + fs gather          [SP/Pool/DMA]
            #   stage1(i): prod, fold, exp               [DVE, Act]
            #   stage2(i): den, wa, agg, scatter         [DVE, Pool]
            # Issue order per iteration: s0(i+1), s1(i), s2(i-1) so the DVE
            # never head-of-line blocks on a gather or on the Act exp.
            live = {}

            def s0(i):
                t0, b, K, off = batches[i]
                bK = b * K
                if itall is not None:
                    it = itall[:, off // 16 : off // 16 + 8 * bK]
                else:
                    itt = epi.tile([128, 8 * BATCH_CAP], I16, tag="srcidx")
                    nc.sync.dma_start(
                        out=itt[:, : 8 * bK],
                        in_=ins[srcname][:, off // 16 : off // 16 + 8 * bK],
                    )
                    it = itt[:, : 8 * bK]
                fsg = epf.tile([128, BATCH_CAP, 128], BF16, tag="fsg")
                nc.gpsimd.dma_gather(
                    out_ap=fsg[:, :bK, :],
                    in_ap=ftab,
                    idxs_ap=it,
                    num_idxs=128 * bK,
                    num_idxs_reg=128 * bK,
                    elem_size=128,
                    single_packet=False,
                )
                live[i] = {"fsg": fsg}

            def s1(i):
                t0, b, K, off = batches[i]
                bK = b * K
                fsg = live[i]["fsg"]
                prod = eps.tile([128, BATCH_CAP, D1], BF16, tag="pw")
                nc.vector.tensor_tensor(
                    out=prod[:, :bK, :D].rearrange("p (j k) d -> p j k d", j=b),
                    in0=fsg[:, :bK, :D].rearrange("p (j k) d -> p j k d", j=b),
                    in1=fdt[:, t0 : t0 + b, :D]
                    .unsqueeze(2)
                    .broadcast_to([128, b, K, D]),
                    op=OP.mult,
                )
                cur = D
                while cur > H:
                    half = cur // 2
                    nc.vector.tensor_tensor(
                        out=prod[:, :bK, :half],
                        in0=prod[:, :bK, :half],
                        in1=prod[:, :bK, half:cur],
                        op=OP.add,
                    )
                    cur = half
                ex = eps.tile([128, BATCH_CAP * HEADS], BF16, tag="ex")
                exv = ex[:, : bK * H].rearrange("p (k h) -> p k h", k=bK)
                nc.scalar.activation(exv, prod[:, :bK, :H], AF.Exp, scale=scale)
                live[i]["ex"] = ex

            def s2(i):
                t0, b, K, off = batches[i]
                bK = b * K
                fsg = live[i]["fsg"]
                ex = live[i]["ex"]
                if layer == 1:
                    stg = eps.tile([128, 4 * 192], F32, tag="stg")
                else:
                    stg = eps.tile([128, 4 * 128], BF16, tag="stg2")
                stg4 = stg[:, : b * stg_w].rearrange("p (b w) -> p b w", b=b)
                if i < 2:
                    # zero the pad columns once per rotating buffer; later
                    # batches never write them so they stay zero
                    nc.gpsimd.memset(
                        stg[:, :].rearrange("p (b w) -> p b w", w=stg_w)[
                            :, :, den0 + H :
                        ],
                        0.0,
                    )
                with nc.allow_low_precision(reason="bf16 hn2 staging"):
                    nc.vector.tensor_reduce(
                        out=stg4[:, :, den0 : den0 + H],
                        in_=ex[:, : bK * H].rearrange(
                            "p (b k h) -> p b h k", b=b, k=K
                        ),
                        axis=AX.X,
                        op=OP.add,
                    )
                nc.vector.tensor_tensor(
                    out=stg4[:, :, den0 : den0 + H],
                    in0=stg4[:, :, den0 : den0 + H],
                    in1=npt[:, t0 : t0 + b].unsqueeze(2).broadcast_to([128, b, H]),
                    op=OP.subtract,
                )
                # weighted features in-place: fsg *= ex (broadcast over c).
                # Split ~1/4 of the slots onto the Pool engine to offload DVE.
                if H > 1:
                    kp = (bK // 4) & ~1
                    if kp >= 8:
                        nc.gpsimd.tensor_tensor(
                            out=fsg[:, :kp, :D].rearrange(
                                "p k (c h) -> p k c h", c=C
                            ),
                            in0=fsg[:, :kp, :D].rearrange(
                                "p k (c h) -> p k c h", c=C
                            ),
                            in1=ex[:, : kp * H]
                            .rearrange("p (k h) -> p k h", k=kp)
                            .unsqueeze(2)
                            .broadcast_to([128, kp, C, H]),
                            op=OP.mult,
                        )
                    else:
                        kp = 0
                    nc.vector.tensor_tensor(
                        out=fsg[:, kp:bK, :D].rearrange(
                            "p k (c h) -> p k c h", c=C
                        ),
                        in0=fsg[:, kp:bK, :D].rearrange(
                            "p k (c h) -> p k c h", c=C
                        ),
                        in1=ex[:, kp * H : bK * H]
                        .rearrange("p (k h) -> p k h", k=bK - kp)
                        .unsqueeze(2)
                        .broadcast_to([128, bK - kp, C, H]),
                        op=OP.mult,
                    )
                else:
                    nc.vector.tensor_tensor(
                        out=fsg[:, :bK, :D],
                        in0=fsg[:, :bK, :D],
                        in1=ex[:, :bK].unsqueeze(2).broadcast_to([128, bK, D]),
                        op=OP.mult,
                    )
                if layer == 1:
                    # aggregation over k on the PE: transpose-accumulate each
                    # weighted slot into PSUM (f32), then transpose back.
                    aggT = aggps.tile([128, 512], F32, name="aggT")
                    for j in range(b):
                        for k in range(K):
                            nc.tensor.matmul(
                                aggT[:, j * 128 : (j + 1) * 128],
                                fsg[:, j * K + k, :],
                                idt[:, :],
                                start=(k == 0),
                                stop=(k == K - 1),
                            )
                    aggTs = eps.tile([128, 512], BF16, tag="aggTs")
                    nc.scalar.activation(
                        aggTs[:, : b * 128], aggT[:, : b * 128], AF.Copy
                    )
                    nodeT = aggps.tile([128, 512], F32, name="nodeT")
                    for j in range(b):
                        nc.tensor.matmul(
                            nodeT[:, j * 128 : (j + 1) * 128],
                            aggTs[:, j * 128 : (j + 1) * 128],
                            idt[:, :],
                            start=True,
                            stop=True,
                        )
                    nc.scalar.activation(
                        stg4[:, :, :D],
                        nodeT[:, : b * 128].rearrange("p (b d) -> p b d", b=b),
                        AF.Copy,
                    )
                else:
                    # aggregation over k: fold (bf16) then reduce (f32 accum)
                    v = fsg[:, :bK, :D].rearrange("p (b k) d -> p b k d", b=b)
                    cur = K
                    folds = 0
                    while folds < 2 and cur > 4:
                        if cur % 2:
                            nc.vector.tensor_tensor(
                                out=v[:, :, 0:1, :D],
                                in0=v[:, :, 0:1, :D],
                                in1=v[:, :, cur - 1 : cur, :D],
                                op=OP.add,
                            )
                            cur -= 1
                        h2 = cur // 2
                        nc.vector.tensor_tensor(
                            out=v[:, :, :h2, :D],
                            in0=v[:, :, :h2, :D],
                            in1=v[:, :, h2:cur, :D],
                            op=OP.add,
                        )
                        cur = h2
                        folds += 1
                    with nc.allow_low_precision(reason="bf16 hn2 staging"):
                        nc.vector.tensor_reduce(
                            out=stg4[:, :, :D],
                            in_=v[:, :, :cur, :D].rearrange("p b k d -> p b d k"),
                            axis=AX.X,
                            op=OP.add,
                        )
                nc.gpsimd.dma_scatter_add(
                    out_ap=hntab[:, :stg_w],
                    in_ap=stg4[:, :, :],
                    idxs_ap=sci[:, 8 * t0 : 8 * (t0 + b)],
                    num_idxs=128 * b,
                    num_idxs_reg=128 * b,
                    elem_size=stg_w,
                    single_packet=False,
                )
                del live[i]

            nbat = len(batches)
            s0(0)
            if nbat > 1:
                s0(1)
            for i in range(nbat):
                if i + 2 < nbat:
                    s0(i + 2)
                s1(i)
                if i >= 1:
                    s2(i - 1)
            s2(nbat - 1)

        # ---------------- merge1 + local W2 projection ----------------
        def merge1(mp, mps, mb_lo, mb_hi):
            for mb in range(mb_lo, mb_hi, MERGE_B):
                b = min(MERGE_B, NT - mb)
                g = mp.tile([128, MERGE_B, 136], F32, tag="g1", name="g1")
                nc.sync.dma_start(
                    out=g[:, :b, :],
                    in_=hn1[:, :136].rearrange("(t p) w -> p t w", p=128)[
                        :, mb : mb + b, :
                    ],
                )
                den = mp.tile([128, MERGE_B * 8], F32, tag="den")
                nc.vector.tensor_scalar_max(den[:, : b * 8], g[:, :b, 128:136], 1e-9)
                rcp = mp.tile([128, MERGE_B * 8], F32, tag="rcp")
                nc.vector.reciprocal(rcp[:, : b * 8], den[:, : b * 8])
                o1 = mp.tile([128, MERGE_B, 128], F32, tag="o1")
                nc.vector.tensor_tensor(
                    out=o1[:, :b, :].rearrange("p b (c h) -> p b c h", c=HID),
                    in0=g[:, :b, :128].rearrange("p b (c h) -> p b c h", c=HID),
                    in1=rcp[:, : b * 8]
                    .rearrange("p (b h) -> p b h", b=b)
                    .unsqueeze(2)
                    .broadcast_to([128, b, HID, HEADS]),
                    op=OP.mult,
                )
                # ELU -> bf16 (2-byte ops get 2x DVE throughput)
                mx = mp.tile([128, MERGE_B, 128], BF16, tag="mx")
                nc.vector.tensor_scalar_max(mx[:, :b, :], o1[:, :b, :], 0.0)
                mn = mp.tile([128, MERGE_B, 128], BF16, tag="mn")
                nc.vector.tensor_scalar_min(mn[:, :b, :], o1[:, :b, :], 0.0)
                e = mp.tile([128, MERGE_B, 128], BF16, tag="e")
                nc.scalar.activation(e[:, :b, :], mn[:, :b, :], AF.Exp)
                nc.vector.tensor_scalar_add(e[:, :b, :], e[:, :b, :], -1.0)
                h1t = mp.tile([128, MERGE_B, 128], BF16, tag="h1t")
                nc.vector.tensor_tensor(
                    out=h1t[:, :b, :], in0=mx[:, :b, :], in1=e[:, :b, :], op=OP.add
                )
                # fused projection: f2 = h1 @ W2 per 128-node tile
                f2p = mp.tile([128, MERGE_B, 128], BF16, tag="f2p")
                nc.gpsimd.memset(f2p[:, :b, D2:], 0.0)
                for j in range(b):
                    psT = mps.tile([128, 128], BF16, name="psT")
                    nc.tensor.matmul(
                        psT[:, :], h1t[:, j, :], idt[:, :],
                        start=True, stop=True, is_transpose=True,
                    )
                    h1T = mp.tile([128, 128], BF16, tag="h1T")
                    nc.scalar.activation(h1T[:, :], psT[:, :], AF.Copy)
                    ps2 = mps.tile([128, D2], F32, name="ps2")
                    nc.tensor.matmul(
                        ps2[:, :], h1T[:, :], w2t[:, :], start=True, stop=True
                    )
                    nc.scalar.activation(f2p[:, j, :D2], ps2[:, :], AF.Copy)
                nc.sync.dma_start(
                    out=f2my[:, :].rearrange("(t p) c -> p t c", p=128)[
                        :, mb : mb + b, :
                    ],
                    in_=f2p[:, :b, :],
                )
                nc.sync.dma_start(
                    out=f2cmp[:, :].rearrange("(t p) c -> p t c", p=128)[
                        :, mb : mb + b, :
                    ],
                    in_=f2p[:, :b, :D2],
                )

        # ---------------- merge2 -> output ----------------
        def merge2(mp):
            for mb in range(0, NT, MERGE_B):
                b = min(MERGE_B, NT - mb)
                g = mp.tile([128, MERGE_B, 65], BF16, tag="g2", name="g2")
                nc.sync.dma_start(
                    out=g[:, :b, :],
                    in_=hn2[:, :65].rearrange("(t p) w -> p t w", p=128)[
                        :, mb : mb + b, :
                    ],
                )
                den = mp.tile([128, MERGE_B], F32, tag="den2")
                nc.vector.tensor_scalar_max(den[:, :b], g[:, :b, 64:65], 1e-9)
                rcp = mp.tile([128, MERGE_B], F32, tag="rcp2")
                nc.vector.reciprocal(rcp[:, :b], den[:, :b])
                o2 = mp.tile([128, MERGE_B, D2], F32, tag="o2")
                nc.vector.tensor_tensor(
                    out=o2[:, :b, :],
                    in0=g[:, :b, :D2],
                    in1=rcp[:, :b].unsqueeze(2).broadcast_to([128, b, D2]),
                    op=OP.mult,
                )
                nc.sync.dma_start(
                    out=out[:, :].rearrange("(t p) c -> p t c", p=128)[
                        :, mb : mb + b, :
                    ],
                    in_=o2[:, :b, :],
                )

        # ---------------- program ----------------
        nb0 = (N + 1023) // 1024  # 49
        split_t = CHUNK // 1024  # 24 (straddling block)
        ep = ctx.enter_context(tc.tile_pool(name="ep", bufs=1))
        eps = ctx.enter_context(tc.tile_pool(name="eps", bufs=2))
        epf = ctx.enter_context(tc.tile_pool(name="epf", bufs=4))
        epi = ctx.enter_context(tc.tile_pool(name="epi", bufs=3))

        with tc.tile_pool(name="p0", bufs=3) as p0, tc.tile_pool(
            name="p0ps", bufs=4, space="PSUM"
        ) as p0ps:
            for t in range(split_t + 1):  # chunk 0 (and straddle)
                phase0_block(t, p0, p0ps)
            for t in range(split_t + 1, nb0):  # chunk 1
                phase0_block(t, p0, p0ps)

        aggps = ctx.enter_context(tc.tile_pool(name="aggps", bufs=2, space="PSUM"))
        mp = ctx.enter_context(tc.tile_pool(name="mp", bufs=2))
        mps = ctx.enter_context(tc.tile_pool(name="mps", bufs=2, space="PSUM"))

        zero_table(hn1, 136, ztile)  # cols 136:192 are never read
        st10 = edge_prep(1, 0, ep)
        st11 = edge_prep(1, 1, ep)
        edge_batches(1, 0, st10, eps, epf, epi, aggps)
        edge_batches(1, 1, st11, eps, epf, epi, aggps)
        merge1(mp, mps, 0, NT)
        # L2 prep only needs local data -> overlaps the collective
        zero_table(hn2, 128, ztile_bf)
        st20 = edge_prep(2, 0, ep)
        st21 = edge_prep(2, 1, ep)
        nc.gpsimd.collective_compute(
            "AllGather",
            OP.bypass,
            ins=[f2cmp[:, :].opt()],
            outs=[f2gath[:, :].opt()],
            replica_groups=[list(range(NCORES))],
        )
        for hh in (0, 1):
            nc.sync.dma_start(
                out=f2tab[hh][:, :D2],
                in_=f2gath[HALF2 * hh : HALF2 * (hh + 1), :],
            )
        edge_batches(2, 0, st20, eps, epf, epi, aggps)
        edge_batches(2, 1, st21, eps, epf, epi, aggps)
        merge2(mp)

    nc.compile()
    return nc


_PROG_CACHE = {}


def _build_cached(sched):
    key = tuple(
        (h, t0, b, K)
        for h in (0, 1)
        for (t0, b, K, off) in sched[h]["batches"]
    )
    if key not in _PROG_CACHE:
        _PROG_CACHE[key] = build(sched)
    return _PROG_CACHE[key]


def _make_in_maps(h, W1, W2, sched, percore):
    hTb = np.ascontiguousarray(np.asarray(h, np.float32).T).astype(bf)
    perm = np.array([(j % 8) * 16 + j // 8 for j in range(128)])
    W1b = np.asarray(W1, np.float32)[:, perm].astype(bf)
    W2b = np.asarray(W2, np.float32)[perm, :].astype(bf)
    ident = np.eye(128, dtype=bf)
    maps = []
    for c in range(NCORES):
        m = {"hTb": hTb, "W1": W1b, "W2b": W2b, "ident": ident}
        for hh in range(2):
            d = percore[c][f"h{hh}"]
            m[f"src1_{hh}"] = wrap16(d["src1"])
            m[f"src2_{hh}"] = wrap16(d["src2"])
            m[f"fd0_{hh}"] = wrap16(d["fd0"])
            m[f"fd1_{hh}"] = wrap16(d["fd1"])
            m[f"fdl_{hh}"] = wrap16(d["fdl"])
            m[f"sct_{hh}"] = wrap16(d["sct"])
            m[f"npad_{hh}"] = d["npad"].astype(np.float32)
        maps.append(m)
    return maps


def kernel(h, W1, W2, src, dst):
    from concourse.bass_utils import run_bass_kernel_spmd

    sched, percore = prep(src, dst)
    nc = _build_cached(sched)
    maps = _make_in_maps(h, W1, W2, sched, percore)
    res = run_bass_kernel_spmd(nc, maps, list(range(NCORES))).results
    out = np.concatenate([res[c]["out"][:NPC] for c in range(NCORES)], axis=0)
    return np.ascontiguousarray(out.astype(np.float32))
